# revision 1
# baseline (speedup 1.0000x reference)
"""LocallyConnected2d Bass kernel for 8 Trainium2 NeuronCores.

Problem (hardcoded): x[16,32,64,64] f32, weight[64,64,32,32,3,3] f32,
bias[32,64,64] f32 -> out[16,32,64,64] f32.  stride=1, pad=1, dil=1.

Sharding: outH split across 8 cores (8 rows each).  Per core, per output
row h: 64 w-positions x 3 kernel-rows of matmuls [K<=97,M=32]x[K,N=16]
accumulated in PSUM.  K = (kernel-col j)*32 + inC c, with a 97th "ones"
row carrying the bias.  Weights/x are cast to bf16 on host (f32 PSUM
accumulation); all SBUF layouts are precomputed host-side so the device
only does plain contiguous DMAs.

w-positions are processed in quads: position w = q*4+g is computed by a
matmul col-tiled to column group g (tile_position=(0,32g)), so the four
LDWEIGHTS+MATMUL streams of a quad run concurrently in the PE array.
PSUM tile is [128 = 4w x 32o, 16 quads x 16b] per output row.
"""

import numpy as np
import ml_dtypes

B, C, H, W = 16, 32, 64, 64
OC = 32
KH = KW = 3
NCORES = 8
RPC = H // NCORES  # rows per core = 8
WCH = 16  # w-positions per weight chunk
NQ = 4  # quad size (PE col groups)

BF16 = ml_dtypes.bfloat16

# x tile chunking by padded row hh: chunk -> (hh0, hh1)
XCHUNKS = [(0, 3), (3, 7), (7, 10)]

_cache = {}


def _build_nc():
    import concourse.bass as bass
    import concourse.tile as tile
    from concourse import bacc, mybir

    nc = bacc.Bacc(
        "TRN2", target_bir_lowering=False, debug=False, num_devices=NCORES
    )
    f32 = mybir.dt.float32
    f16 = mybir.dt.float16
    bf16 = mybir.dt.bfloat16

    # xs: [97, 10, 64*16] bf16.  Partition j*32+c holds x[c, hh, w+j, b]
    # (hh = local padded row 0..9, w = out col, b = batch); row 96 = 1.0.
    xs = nc.dram_tensor("xs", (97, 10, W * B), bf16, kind="ExternalInput")
    # wt: [8, 4, 97, 16*3*32] bf16; [h, chunk, j*32+c, (wl*3+ik)*32+o];
    # row 96 holds bias at ik==2 slots, zeros elsewhere.
    wt = nc.dram_tensor(
        "wt", (RPC, W // WCH, 97, WCH * KH * OC), bf16, kind="ExternalInput"
    )
    # out: [8, 128, 16*16] f16 = out[h, g*32+o, q*16+b] with w = q*4+g
    out = nc.dram_tensor(
        "out", (RPC, 4 * OC, (W // NQ) * B), f16, kind="ExternalOutput"
    )

    with tile.TileContext(nc) as tc:
        with (
            tc.tile_pool(name="xpool", bufs=1) as xpool,
            tc.tile_pool(name="wpool", bufs=6) as wpool,
            tc.tile_pool(name="opool", bufs=4) as opool,
            tc.tile_pool(name="psum", bufs=3, space="PSUM") as ppool,
        ):
            # x row-chunks as separate tiles for fine-grained DMA deps.
            # x rides the scalar HWDGE ring so weight DMAs (sync ring)
            # stream in parallel from t=0.
            xtiles = []
            for ci, (h0, h1) in enumerate(XCHUNKS):
                t = xpool.tile([97, h1 - h0, W * B], bf16, tag=f"x{ci}")
                nc.scalar.dma_start(t[:], xs[:, h0:h1])
                xtiles.append(t)

            def xslice(hh, w, k):
                for (h0, h1), t in zip(XCHUNKS, xtiles):
                    if h0 <= hh < h1:
                        return t[0:k, hh - h0, w * B : (w + 1) * B]
                raise AssertionError

            for h in range(RPC):
                wtiles = []
                for ch in range(W // WCH):
                    wti = wpool.tile([97, WCH * KH * OC], bf16)
                    nc.sync.dma_start(wti[:], wt[h, ch])
                    wtiles.append(wti)

                pt = ppool.tile([4 * OC, (W // NQ) * B], f32)
                for q in range(W // NQ):
                    for g in range(NQ):
                        w = q * NQ + g
                        wti = wtiles[w // WCH]
                        wl = w % WCH
                        for ik in range(KH):
                            k = 97 if ik == 2 else 96
                            woff = (wl * 3 + ik) * 32
                            nc.tensor.matmul(
                                pt[32 * g : 32 * (g + 1), q * B : (q + 1) * B],
                                wti[0:k, woff : woff + 32],
                                xslice(h + ik, w, k),
                                start=(ik == 0),
                                stop=(ik == 2),
                                tile_position=(0, 32 * g),
                            )
                ot = opool.tile([4 * OC, (W // NQ) * B], f16)
                nc.vector.tensor_copy(ot[:], pt[:])
                nc.scalar.dma_start(out[h], ot[:])
    nc.compile()
    return nc


def _prep_inputs(x, weight, bias):
    """Host-side shard + layout prep.  Returns list of 8 per-core dicts."""
    # padded x, transposed to [c, hh, wp, b]
    xp = np.zeros((C, H + 2, W + 2, B), dtype=BF16)
    xp[:, 1 : H + 1, 1 : W + 1, :] = np.ascontiguousarray(
        x.transpose(1, 2, 3, 0)
    ).astype(BF16)

    # weight -> [h, j, c, w, ik, o]
    wtr = np.ascontiguousarray(weight.transpose(0, 5, 3, 1, 4, 2)).astype(BF16)
    wtr = wtr.reshape(H, 96, W, KH, OC)
    btr = bias.transpose(1, 2, 0).astype(BF16)  # [h, w, o]

    in_maps = []
    for i in range(NCORES):
        h0 = i * RPC
        # xs replication: [97, 10, 64, 16]
        rep = np.zeros((97, RPC + 2, W, B), dtype=BF16)
        slab = xp[:, h0 : h0 + RPC + 2, :, :]  # [32, 10, 66, 16]
        for j in range(KW):
            rep[j * 32 : (j + 1) * 32] = slab[:, :, j : j + W, :]
        rep[96] = 1.0

        wcore = np.zeros((RPC, 97, W, KH, OC), dtype=BF16)
        wcore[:, 0:96] = wtr[h0 : h0 + RPC]
        wcore[:, 96, :, 2, :] = btr[h0 : h0 + RPC]  # bias via ones-row, ik==2
        # -> [h, chunk, 97, WCH*3*32]
        wcore = wcore.reshape(RPC, 97, W // WCH, WCH * KH * OC).transpose(
            0, 2, 1, 3
        )

        in_maps.append(
            {
                "xs": np.ascontiguousarray(rep.reshape(97, RPC + 2, W * B)),
                "wt": np.ascontiguousarray(wcore),
            }
        )
    return in_maps


def _run(in_maps, trace=False, tmpdir=None):
    from concourse.bass_utils import run_bass_kernel_spmd

    if "nc" not in _cache:
        _cache["nc"] = _build_nc()
    return run_bass_kernel_spmd(
        _cache["nc"], in_maps, list(range(NCORES)), trace=trace, tmpdir=tmpdir
    )


def _assemble(results):
    out = np.empty((B, OC, H, W), dtype=np.float32)
    for i in range(NCORES):
        # res: [h, g*32+o, q*16+b], w = q*4+g
        res = (
            results[i]["out"].astype(np.float32).reshape(RPC, NQ, OC, W // NQ, B)
        )
        # -> out[b, o, h, q*4+g]
        out[:, :, i * RPC : (i + 1) * RPC, :] = res.transpose(
            4, 2, 0, 3, 1
        ).reshape(B, OC, RPC, W)
    return out


def kernel(x, weight, bias):
    x = np.asarray(x)
    weight = np.asarray(weight)
    bias = np.asarray(bias)
    in_maps = _prep_inputs(x, weight, bias)
    results = _run(in_maps).results
    return _assemble(results)



# revision 2
# speedup vs baseline: 1.4558x; 1.4558x over previous
"""LocallyConnected2d Bass kernel for 8 Trainium2 NeuronCores.

Problem (hardcoded): x[16,32,64,64] f32, weight[64,64,32,32,3,3] f32,
bias[32,64,64] f32 -> out[16,32,64,64] f32.  stride=1, pad=1, dil=1.

Sharding: outH split across 8 cores (8 rows each).  Per core, per output
row h: 64 w-positions x 3 kernel-rows of matmuls [K<=97,M=32]x[K,N=16]
accumulated in PSUM.  K = (kernel-col j)*32 + inC c, with a 97th "ones"
row carrying the bias.

Weights are stored fp8 e3m4 (scaled by 2^8 on host; output descaled by
2^-8 on host - exact), x is bf16, PSUM accumulates f32.  This halves the
dominant weight HBM traffic.  One weight DMA per output row keeps the
HWDGE descriptor-generation overhead (~625ns/DMA) off the critical path.

w-positions are processed in quads: position w = q*4+g is computed by a
matmul col-tiled to column group g (tile_position=(0,32g)), so the four
LDWEIGHTS+MATMUL streams of a quad run concurrently in the PE array.
PSUM tile is [128 = 4w x 32o, 16 quads x 16b] per output row.
"""

import numpy as np
import ml_dtypes

B, C, H, W = 16, 32, 64, 64
OC = 32
KH = KW = 3
NCORES = 8
RPC = H // NCORES  # rows per core = 8
NQ = 4  # quad size (PE col groups)
WSCALE = 2.0**8  # weight scale into fp8e3 range (max 15.08 < 15.5)

BF16 = ml_dtypes.bfloat16
F8E3 = ml_dtypes.float8_e3m4

# x tile chunking by padded row hh: chunk -> (hh0, hh1)
XCHUNKS = [(0, 3), (3, 7), (7, 10)]

_cache = {}


def _build_nc():
    import concourse.bass as bass
    import concourse.tile as tile
    from concourse import bacc, mybir

    nc = bacc.Bacc(
        "TRN2", target_bir_lowering=False, debug=False, num_devices=NCORES
    )
    f32 = mybir.dt.float32
    f16 = mybir.dt.float16
    bf16 = mybir.dt.bfloat16
    f8e3 = mybir.dt.float8e3

    # xs: [97, 10, 64*16] bf16.  Partition j*32+c holds x[c, hh, w+j, b]
    # (hh = local padded row 0..9, w = out col, b = batch); row 96 = 1.0.
    xs = nc.dram_tensor("xs", (97, 10, W * B), bf16, kind="ExternalInput")
    # wt: [8, 97, 64*3*32] f8e3; [h, j*32+c, (w*3+ik)*32+o] scaled by 2^8;
    # row 96 holds bias*2^8 at ik==2 slots, zeros elsewhere.
    wt = nc.dram_tensor(
        "wt", (RPC, 97, W * KH * OC), f8e3, kind="ExternalInput"
    )
    # out: [8, 128, 16*16] f16 = 2^8 * out[h, g*32+o, q*16+b] with w = q*4+g
    out = nc.dram_tensor(
        "out", (RPC, 4 * OC, (W // NQ) * B), f16, kind="ExternalOutput"
    )

    with tile.TileContext(nc) as tc:
        with (
            tc.tile_pool(name="xpool", bufs=1) as xpool,
            tc.tile_pool(name="wpool", bufs=8) as wpool,
            tc.tile_pool(name="opool", bufs=4) as opool,
            tc.tile_pool(name="psum", bufs=3, space="PSUM") as ppool,
        ):
            # x row-chunks as separate tiles for fine-grained DMA deps.
            # x rides the scalar HWDGE ring so weight DMAs (sync ring)
            # stream in parallel from t=0.
            xtiles = []
            for ci, (h0, h1) in enumerate(XCHUNKS):
                t = xpool.tile([97, h1 - h0, W * B], bf16, tag=f"x{ci}")
                nc.scalar.dma_start(t[:], xs[:, h0:h1])
                xtiles.append(t)

            def xslice(hh, w, k):
                for (h0, h1), t in zip(XCHUNKS, xtiles):
                    if h0 <= hh < h1:
                        return t[0:k, hh - h0, w * B : (w + 1) * B]
                raise AssertionError

            wtiles = []
            for h in range(RPC):
                wti = wpool.tile([97, W * KH * OC], f8e3)
                nc.sync.dma_start(wti[:], wt[h])
                wtiles.append(wti)

            for h in range(RPC):
                wti = wtiles[h]
                pt = ppool.tile([4 * OC, (W // NQ) * B], f32)
                for q in range(W // NQ):
                    for g in range(NQ):
                        w = q * NQ + g
                        for ik in range(KH):
                            k = 97 if ik == 2 else 96
                            woff = (w * 3 + ik) * 32
                            nc.tensor.matmul(
                                pt[32 * g : 32 * (g + 1), q * B : (q + 1) * B],
                                wti[0:k, woff : woff + 32],
                                xslice(h + ik, w, k),
                                start=(ik == 0),
                                stop=(ik == 2),
                                tile_position=(0, 32 * g),
                            )
                ot = opool.tile([4 * OC, (W // NQ) * B], f16)
                nc.vector.tensor_copy(ot[:], pt[:])
                nc.scalar.dma_start(out[h], ot[:])
    nc.compile()
    return nc


def _prep_inputs(x, weight, bias):
    """Host-side shard + layout prep.  Returns list of 8 per-core dicts."""
    # padded x, transposed to [c, hh, wp, b]
    xp = np.zeros((C, H + 2, W + 2, B), dtype=BF16)
    xp[:, 1 : H + 1, 1 : W + 1, :] = np.ascontiguousarray(
        x.transpose(1, 2, 3, 0)
    ).astype(BF16)

    # weight -> [h, j, c, w, ik, o], scaled into fp8e3 range
    wtr = np.ascontiguousarray(
        weight.transpose(0, 5, 3, 1, 4, 2) * np.float32(WSCALE)
    ).astype(F8E3)
    wtr = wtr.reshape(H, 96, W, KH, OC)
    btr = (bias.transpose(1, 2, 0) * np.float32(WSCALE)).astype(F8E3)  # [h,w,o]

    in_maps = []
    for i in range(NCORES):
        h0 = i * RPC
        # xs replication: [97, 10, 64, 16]
        rep = np.zeros((97, RPC + 2, W, B), dtype=BF16)
        slab = xp[:, h0 : h0 + RPC + 2, :, :]  # [32, 10, 66, 16]
        for j in range(KW):
            rep[j * 32 : (j + 1) * 32] = slab[:, :, j : j + W, :]
        rep[96] = 1.0

        wcore = np.zeros((RPC, 97, W, KH, OC), dtype=F8E3)
        wcore[:, 0:96] = wtr[h0 : h0 + RPC]
        wcore[:, 96, :, 2, :] = btr[h0 : h0 + RPC]  # bias via ones-row, ik==2

        in_maps.append(
            {
                "xs": np.ascontiguousarray(rep.reshape(97, RPC + 2, W * B)),
                "wt": np.ascontiguousarray(
                    wcore.reshape(RPC, 97, W * KH * OC)
                ),
            }
        )
    return in_maps


def _run(in_maps, trace=False, tmpdir=None):
    from concourse.bass_utils import run_bass_kernel_spmd

    if "nc" not in _cache:
        _cache["nc"] = _build_nc()
    return run_bass_kernel_spmd(
        _cache["nc"], in_maps, list(range(NCORES)), trace=trace, tmpdir=tmpdir
    )


def _assemble(results):
    out = np.empty((B, OC, H, W), dtype=np.float32)
    inv = np.float32(1.0 / WSCALE)
    for i in range(NCORES):
        # res: [h, g*32+o, q*16+b], w = q*4+g
        res = (
            results[i]["out"].astype(np.float32).reshape(RPC, NQ, OC, W // NQ, B)
            * inv
        )
        # -> out[b, o, h, q*4+g]
        out[:, :, i * RPC : (i + 1) * RPC, :] = res.transpose(
            4, 2, 0, 3, 1
        ).reshape(B, OC, RPC, W)
    return out


def kernel(x, weight, bias):
    x = np.asarray(x)
    weight = np.asarray(weight)
    bias = np.asarray(bias)
    in_maps = _prep_inputs(x, weight, bias)
    results = _run(in_maps).results
    return _assemble(results)


# revision 5
# speedup vs baseline: 1.6093x; 1.1054x over previous
"""LocallyConnected2d Bass kernel for 8 Trainium2 NeuronCores.

Problem (hardcoded): x[16,32,64,64] f32, weight[64,64,32,32,3,3] f32,
bias[32,64,64] f32 -> out[16,32,64,64] f32.  stride=1, pad=1, dil=1.

Sharding: outH split across 8 cores (8 rows each).  Per core, per output
row h: 64 w-positions x 3 kernel-rows of matmuls [K<=97,M=32]x[K,N=16]
accumulated in PSUM.  K = (kernel-col j)*32 + inC c, with a 97th "ones"
row carrying the bias.

Traffic optimizations vs the bf16 baseline:
  - weights (the dominant stream) stored fp8 e3m4, scaled by 2^8 on host
    (output descaled by 2^-8 on host - exact).  Halves weight HBM bytes.
  - x is DMAed once (unreplicated, [32c, 10hh, 66wp*16b] bf16) into
    partitions 0..31; the kernel-column-shifted copies for partition
    groups 1,2 (j=1,2) are made on-chip by DVE partition-shifted copies
    (4x perf mode), cutting x HBM bytes 3x.
  - one weight DMA per output row (row 7 in two halves to shorten the
    dependent tail); out DMAs issued on the sync ring after all weight
    DMAs so weights win the DMA-engine queue.

w-positions are processed in quads: position w = q*4+g is computed by a
matmul col-tiled to column group g (tile_position=(0,32g)), so the four
LDWEIGHTS+MATMUL streams of a quad run concurrently in the PE array.
PSUM tile is [128 = 4w x 32o, 16 quads x 16b] per output row.
"""

import numpy as np
import ml_dtypes

B, C, H, W = 16, 32, 64, 64
OC = 32
KH = KW = 3
NCORES = 8
RPC = H // NCORES  # rows per core = 8
NQ = 4  # quad size (PE col groups)
WSCALE = 2.0**8  # weight scale into fp8e3 range (max 15.08 < 15.5)
WP = W + 2  # padded width positions per row

BF16 = ml_dtypes.bfloat16
F8E3 = ml_dtypes.float8_e3m4

# x tile chunking by padded row hh: chunk -> (hh0, hh1)
XCHUNKS = [(0, 3), (3, 7), (7, 10)]

_cache = {}


def _build_nc():
    import concourse.bass as bass
    import concourse.tile as tile
    from concourse import bacc, mybir

    nc = bacc.Bacc(
        "TRN2", target_bir_lowering=False, debug=False, num_devices=NCORES
    )
    f32 = mybir.dt.float32
    f16 = mybir.dt.float16
    bf16 = mybir.dt.bfloat16
    f8e3 = mybir.dt.float8e3

    # xs: [33, 10, 66*16] bf16.  Partition c<32 holds x[c, hh, wp, b]
    # (hh = local padded row 0..9, wp = padded col 0..65, b = batch);
    # partition 32 is all-ones (bias row source).
    xs = nc.dram_tensor("xs", (33, 10, WP * B), bf16, kind="ExternalInput")
    # wt: [8, 97, 64*3*32] f8e3; [h, j*32+c, (w*3+ik)*32+o] scaled by 2^8;
    # row 96 holds bias*2^8 at ik==2 slots, zeros elsewhere.
    wt = nc.dram_tensor(
        "wt", (RPC, 97, W * KH * OC), f8e3, kind="ExternalInput"
    )
    # out: [8, 128, 16*16] f16 = 2^8 * out[h, g*32+o, q*16+b] with w = q*4+g
    out = nc.dram_tensor(
        "out", (RPC, 4 * OC, (W // NQ) * B), f16, kind="ExternalOutput"
    )

    with tile.TileContext(nc) as tc:
        with (
            tc.tile_pool(name="xpool", bufs=1) as xpool,
            tc.tile_pool(name="wpool", bufs=9) as wpool,
            tc.tile_pool(name="opool", bufs=4) as opool,
            tc.tile_pool(name="psum", bufs=3, space="PSUM") as ppool,
            tc.tile_pool(name="psum7", bufs=1, space="PSUM") as ppool7,
        ):
            # Per-chunk x tiles [97, rows, WP*16].  DMA x once into
            # partitions 0..31 (scalar ring), ones row into 96; DVE makes
            # the j=1,2 column-shifted copies into partitions 32..95.
            xtiles = []
            for ci, (h0, h1) in enumerate(XCHUNKS):
                r = h1 - h0
                t = xpool.tile([97, r, WP * B], bf16, tag=f"x{ci}")
                nc.scalar.dma_start(t[0:32], xs[0:32, h0:h1])
                nc.scalar.dma_start(
                    t[96:97, :, 0 : W * B], xs[32:33, h0:h1, 0 : W * B]
                )
                for j in (1, 2):
                    nc.vector.tensor_copy(
                        t[32 * j : 32 * (j + 1), :, 0 : W * B],
                        t[0:32, :, j * B : j * B + W * B],
                    )
                xtiles.append(t)

            def xslice(hh, w, k):
                for (h0, h1), t in zip(XCHUNKS, xtiles):
                    if h0 <= hh < h1:
                        return t[0:k, hh - h0, w * B : (w + 1) * B]
                raise AssertionError

            # Weight DMAs on the sync ring: rows 0..6 whole, row 7 in two
            # halves so the final dependent chain starts earlier.
            wtiles = []
            for h in range(RPC - 1):
                wti = wpool.tile([97, W * KH * OC], f8e3)
                nc.sync.dma_start(wti[:], wt[h])
                wtiles.append(wti)
            HWC = (W // 2) * KH * OC  # 3072: half-row weight columns
            w7a = wpool.tile([97, HWC], f8e3, tag="w7a")
            nc.sync.dma_start(w7a[:], wt[RPC - 1, :, 0:HWC])
            w7b = wpool.tile([97, HWC], f8e3, tag="w7b")
            nc.sync.dma_start(w7b[:], wt[RPC - 1, :, HWC : 2 * HWC])

            def wslice(h, w, ik, k):
                if h < RPC - 1:
                    return wtiles[h][0:k, (w * 3 + ik) * 32 :][:, 0:32]
                t, wl = (w7a, w) if w < W // 2 else (w7b, w - W // 2)
                return t[0:k, (wl * 3 + ik) * 32 :][:, 0:32]

            outs = []  # (dram slice, sbuf tile) deferred out DMAs

            def do_row(h, pt, q0, q1, osl):
                for q in range(q0, q1):
                    for g in range(NQ):
                        w = q * NQ + g
                        for ik in range(KH):
                            k = 97 if ik == 2 else 96
                            nc.tensor.matmul(
                                pt[
                                    32 * g : 32 * (g + 1),
                                    (q - q0) * B : (q - q0 + 1) * B,
                                ],
                                wslice(h, w, ik, k),
                                xslice(h + ik, w, k),
                                start=(ik == 0),
                                stop=(ik == 2),
                                tile_position=(0, 32 * g),
                            )
                ot = opool.tile([4 * OC, (q1 - q0) * B], f16)
                nc.vector.tensor_copy(ot[:], pt[:])
                outs.append((osl, ot))

            NQW = W // NQ  # 16 quads per row
            for h in range(RPC - 1):
                pt = ppool.tile([4 * OC, NQW * B], f32)
                do_row(h, pt, 0, NQW, out[h])
            # row 7: two half-row PSUM tiles
            pta = ppool7.tile([4 * OC, (NQW // 2) * B], f32, tag="p7a")
            do_row(RPC - 1, pta, 0, NQW // 2, out[RPC - 1, :, 0 : NQW * B // 2])
            ptb = ppool7.tile([4 * OC, (NQW // 2) * B], f32, tag="p7b")
            do_row(
                RPC - 1, ptb, NQW // 2, NQW, out[RPC - 1, :, NQW * B // 2 :]
            )

            # out DMAs on the sync ring, issued after all weight DMAs so
            # weight transfers win the DMA-engine queue.
            for osl, ot in outs:
                nc.sync.dma_start(osl, ot[:])
    nc.compile()
    return nc


def _prep_inputs(x, weight, bias):
    """Host-side shard + layout prep.  Returns list of 8 per-core dicts."""
    # padded x, transposed to [c, hh, wp, b]
    xp = np.zeros((C, H + 2, W + 2, B), dtype=BF16)
    xp[:, 1 : H + 1, 1 : W + 1, :] = np.ascontiguousarray(
        x.transpose(1, 2, 3, 0)
    ).astype(BF16)

    # weight -> [h, j, c, w, ik, o], scaled into fp8e3 range
    wtr = np.ascontiguousarray(
        weight.transpose(0, 5, 3, 1, 4, 2) * np.float32(WSCALE)
    ).astype(F8E3)
    wtr = wtr.reshape(H, 96, W, KH, OC)
    btr = (bias.transpose(1, 2, 0) * np.float32(WSCALE)).astype(F8E3)  # [h,w,o]

    in_maps = []
    for i in range(NCORES):
        h0 = i * RPC
        xcore = np.ones((33, RPC + 2, WP, B), dtype=BF16)
        xcore[0:32] = xp[:, h0 : h0 + RPC + 2, :, :]

        wcore = np.zeros((RPC, 97, W, KH, OC), dtype=F8E3)
        wcore[:, 0:96] = wtr[h0 : h0 + RPC]
        wcore[:, 96, :, 2, :] = btr[h0 : h0 + RPC]  # bias via ones-row, ik==2

        in_maps.append(
            {
                "xs": np.ascontiguousarray(xcore.reshape(33, RPC + 2, WP * B)),
                "wt": np.ascontiguousarray(
                    wcore.reshape(RPC, 97, W * KH * OC)
                ),
            }
        )
    return in_maps


def _run(in_maps, trace=False, tmpdir=None):
    from concourse.bass_utils import run_bass_kernel_spmd

    if "nc" not in _cache:
        _cache["nc"] = _build_nc()
    return run_bass_kernel_spmd(
        _cache["nc"], in_maps, list(range(NCORES)), trace=trace, tmpdir=tmpdir
    )


def _assemble(results):
    out = np.empty((B, OC, H, W), dtype=np.float32)
    inv = np.float32(1.0 / WSCALE)
    for i in range(NCORES):
        # res: [h, g*32+o, q*16+b], w = q*4+g
        res = (
            results[i]["out"].astype(np.float32).reshape(RPC, NQ, OC, W // NQ, B)
            * inv
        )
        # -> out[b, o, h, q*4+g]
        out[:, :, i * RPC : (i + 1) * RPC, :] = res.transpose(
            4, 2, 0, 3, 1
        ).reshape(B, OC, RPC, W)
    return out


def kernel(x, weight, bias):
    x = np.asarray(x)
    weight = np.asarray(weight)
    bias = np.asarray(bias)
    in_maps = _prep_inputs(x, weight, bias)
    results = _run(in_maps).results
    return _assemble(results)
